# revision 50
# baseline (speedup 1.0000x reference)
"""Multi-head attention (B=4, T=2048, E=1024, H=16, D=64) on 8 TRN2 cores.

Sharding: core c handles batch b = c//2 and heads hg = c%2 (8 heads each).
No cross-device comms: each core emits a partial out-projection
y_partial[b] over its 512 head-columns; the host sums core pairs.

v2 design (per core):
  - q/k/v projections: fp8e4 DoubleRow matmuls (2 contraction chunks per
    instruction) except t-block 0 / first 512 keys which use fp16 for
    accuracy where causal attention concentrates on few keys.
  - RoPE: rotate-half via fp16 PE permutation matmul, combine on DVE.
  - QK: scoresT[k,q] per (qb, head, 128-key chunk); q-block 0 in fp16,
    later q-blocks as fp8 DoubleRow with a zeros second subtile.
  - exp on ACT, fused over chunk PAIRS (one activation per 2 chunks),
    output fp8e4 (fp16 for qb 0); causal masking via narrowed
    affine_select windows on GPSIMD.
  - AV transposed: out o[t,d] = e[k,t]^T v[k,d] with a ones column in v
    giving the softmax denominator; fp8 DoubleRow over kc pairs.
  - divide: per-partition reciprocal + broadcast multiply on DVE,
    then DMA-transpose o[t,d] -> oT[d,t] for the out-projection.
  - out-projection in fp16; y emitted fp16, host sums core pairs.
"""
import sys
import numpy as np
from contextlib import ExitStack

try:
    import concourse  # noqa: F401
except ImportError:
    sys.path.insert(0, "/opt/trn_rl_repo")

import concourse.tile as tile  # noqa: E402
from concourse import bacc, mybir  # noqa: E402
from concourse.bass_utils import run_bass_kernel_spmd  # noqa: E402
import ml_dtypes  # noqa: E402

F32 = mybir.dt.float32
F16 = mybir.dt.float16
FP8 = mybir.dt.float8e4
FP8E = mybir.dt.float8e5
AF = mybir.ActivationFunctionType
ALU = mybir.AluOpType
DR = mybir.MatmulPerfMode.DoubleRow

B, T, E, H, D = 4, 2048, 1024, 16, 64
N_CORES = 8
HPC = 8            # heads per core
EC = HPC * D       # 512 head-columns per core
TB = 512           # t/q block
KC = 128           # k chunk
NTB = T // TB      # 4
NTT = T // KC      # 16
CCH = E // 128     # 8 contraction chunks for x projections
OCH = EC // 128    # 4 chunks of the per-core head-column dim
ROPE_BASE = 10000.0

_NC = None
DEBUG_DUMP = False


def _build():
    nc = bacc.Bacc("TRN2", target_bir_lowering=False, debug=False,
                   num_devices=N_CORES)
    ap = {}

    def din(name, shape, dt):
        ap[name] = nc.dram_tensor(name, shape, dt, kind="ExternalInput").ap()

    din("x8", [E, T], FP8)            # x[b].T fp8
    din("x16", [E, TB], F16)          # x[b].T[:, :512] fp16
    din("wq8", [E, EC], FP8)
    din("wk8", [E, EC], FP8)
    din("wv8", [E, EC], FP8)
    din("wq16", [E, EC], F16)
    din("wk16", [E, EC], F16)
    din("wv16", [E, EC], F16)
    din("wo16", [EC, E], F16)
    din("p2t", [128, 128], F16)       # rotate-half permutation (lhsT form)
    din("cosb", [128, T], F16)        # cos dup'd over 2 heads, [2*64, T]
    din("sinb", [128, T], F16)
    y = nc.dram_tensor("y", [T, E], F16, kind="ExternalOutput").ap()

    with tile.TileContext(nc) as tc, ExitStack() as octx:
        persist = octx.enter_context(tc.tile_pool(name="persist", bufs=1))
        x8_sb = persist.tile([128, CCH, T], FP8, tag="x8")
        x16_sb = persist.tile([128, CCH, TB], F16, tag="x16")
        wq8_sb = persist.tile([128, CCH, EC], FP8, tag="wq8")
        wk8_sb = persist.tile([128, CCH, EC], FP8, tag="wk8")
        wv8_sb = persist.tile([128, CCH, EC], FP8, tag="wv8")
        wq16_sb = persist.tile([128, CCH, EC], F16, tag="wq16")
        wk16_sb = persist.tile([128, CCH, EC], F16, tag="wk16")
        wv16_sb = persist.tile([128, CCH, EC], F16, tag="wv16")
        wo_sb = persist.tile([128, OCH, E], F16, tag="wo")
        p2t_sb = persist.tile([128, 128], F16, tag="p2t")
        cos_sb = persist.tile([128, T], F16, tag="cos")
        sin_sb = persist.tile([128, T], F16, tag="sin")
        q16 = persist.tile([128, OCH, TB], F16, tag="q16")
        k16 = persist.tile([128, OCH, TB], F16, tag="k16")
        q8 = persist.tile([128, OCH, T], FP8, tag="q8")
        k8 = persist.tile([128, OCH, 2, T], FP8, tag="k8")  # slot1 = zeros
        vv8 = persist.tile([128, NTT, HPC, D + 1], FP8, tag="vv8")
        vv16 = persist.tile([128, 2 * NTB, HPC, D + 1], F16, tag="vv16")
        oT = persist.tile([128, OCH, T], F16, tag="oT")

        # --------- initial DMAs (startup-critical first) ---------
        def load(dst, src, n=1):
            # split along dim0 chunks of dst's second axis for pipelining
            nc.sync.dma_start(out=dst, in_=src)

        # two HWDGE queues: SP carries x16/wq16/... , ACT carries wk16 and
        # the small rope constants (ACT is idle until the first exp).
        x16r = ap["x16"].rearrange("(c p) t -> p c t", p=128)
        x8r = ap["x8"].rearrange("(c p) t -> p c t", p=128)
        wk16src = ap["wk16"].rearrange("(c p) e -> p c e", p=128)
        nc.scalar.dma_start(out=p2t_sb, in_=ap["p2t"])
        nc.scalar.dma_start(out=cos_sb[:, 0:TB], in_=ap["cosb"][:, 0:TB])
        nc.scalar.dma_start(out=sin_sb[:, 0:TB], in_=ap["sinb"][:, 0:TB])
        for m in range(OCH):
            nc.scalar.dma_start(
                out=wk16_sb[:, :, m * 128:(m + 1) * 128],
                in_=wk16src[:, :, m * 128:(m + 1) * 128])
        nc.scalar.dma_start(out=cos_sb[:, TB:], in_=ap["cosb"][:, TB:])
        nc.scalar.dma_start(out=sin_sb[:, TB:], in_=ap["sinb"][:, TB:])
        nc.sync.dma_start(out=x16_sb, in_=x16r)
        wq16src = ap["wq16"].rearrange("(c p) e -> p c e", p=128)
        for m in range(OCH):
            nc.sync.dma_start(out=wq16_sb[:, :, m * 128:(m + 1) * 128],
                              in_=wq16src[:, :, m * 128:(m + 1) * 128])
        nc.sync.dma_start(
            out=wv16_sb, in_=ap["wv16"].rearrange("(c p) e -> p c e", p=128))
        nc.sync.dma_start(out=x8_sb, in_=x8r)
        for name, dst in (("wk8", wk8_sb), ("wq8", wq8_sb), ("wv8", wv8_sb)):
            nc.sync.dma_start(
                out=dst, in_=ap[name].rearrange("(c p) e -> p c e", p=128))
        nc.sync.dma_start(
            out=wo_sb, in_=ap["wo16"].rearrange("(c p) e -> p c e", p=128))

        # zeros + ones init (Pool; overlaps first projections)
        nc.gpsimd.memset(vv8[:, :, :, D], 1.0)
        nc.gpsimd.memset(vv16[:, :, :, D], 1.0)
        # exp bias: e^(s/8 - 2) keeps fp8 e4m3 in range for extreme scores
        nbias = persist.tile([128, 1], F32, tag="nbias")
        nc.vector.memset(nbias[:], -2.0)

        # --------- pools ---------
        sp_pool = octx.enter_context(
            tc.tile_pool(name="sp", bufs=2, space="PSUM"))    # 4 banks
        op_pool = octx.enter_context(
            tc.tile_pool(name="op", bufs=2, space="PSUM"))    # 2 banks
        sc_pool = octx.enter_context(
            tc.tile_pool(name="sc", bufs=2, space="PSUM"))    # 2 banks
        e8_pool = octx.enter_context(tc.tile_pool(name="e8", bufs=12))
        e16_pool = octx.enter_context(tc.tile_pool(name="e16", bufs=7))
        tmp_pool = octx.enter_context(tc.tile_pool(name="tmp", bufs=8))
        raw_pool = octx.enter_context(tc.tile_pool(name="raw", bufs=5))
        osb_pool = octx.enter_context(tc.tile_pool(name="osb", bufs=4))
        ysb_pool = octx.enter_context(tc.tile_pool(name="ysb", bufs=4))
        rsb_pool = octx.enter_context(tc.tile_pool(name="rsb", bufs=4))

        # --------- projection emitters ---------
        def emit_qk_proj(b, which, ch):
            """Project + rope q or k for t-block b, head-pair chunk ch."""
            ts = slice(b * TB, (b + 1) * TB)
            pp = sc_pool.tile([128, TB], F32, tag="pp")
            if b == 0:
                w = wq16_sb if which == "q" else wk16_sb
                for c in range(CCH):
                    nc.tensor.matmul(
                        pp[:], w[:, c, ch * 128:(ch + 1) * 128],
                        x16_sb[:, c, :], start=(c == 0), stop=(c == CCH - 1))
            else:
                w = wq8_sb if which == "q" else wk8_sb
                for j in range(CCH // 2):
                    nc.tensor.matmul(
                        pp[:], w[:, 2 * j:2 * j + 2, ch * 128:(ch + 1) * 128],
                        x8_sb[:, 2 * j:2 * j + 2, ts],
                        start=(j == 0), stop=(j == CCH // 2 - 1),
                        perf_mode=DR)
            raw = raw_pool.tile([128, TB], F16, tag="raw")
            if b == 0 and ch == 0:
                nc.scalar.copy(raw[:], pp[:])
            else:
                nc.vector.tensor_copy(raw[:], pp[:])
            sw = sc_pool.tile([128, TB], F32, tag="pp")
            nc.tensor.matmul(sw[:], p2t_sb[:], raw[:], start=True, stop=True)
            t1 = tmp_pool.tile([128, TB], F16, tag="t1")
            nc.vector.tensor_mul(t1[:], raw[:], cos_sb[:, ts])
            t2 = tmp_pool.tile([128, TB], F16, tag="t2")
            nc.vector.tensor_mul(t2[:], sw[:], sin_sb[:, ts])
            if b == 0:
                dst16 = q16 if which == "q" else k16
                nc.vector.tensor_add(dst16[:, ch, :], t1[:], t2[:])
                if which == "k":
                    nc.vector.tensor_add(k8[:, ch, 0, 0:TB], t1[:], t2[:])
                    # slot1 = fp16 k minus fp8 k: QK DR adds k_res @ q,
                    # cancelling the k-side quantization error
                    nc.vector.tensor_sub(k8[:, ch, 1, 0:TB],
                                         dst16[:, ch, :], k8[:, ch, 0, 0:TB])
            else:
                if which == "q":
                    nc.vector.tensor_add(q8[:, ch, ts], t1[:], t2[:])
                else:
                    ksum = tmp_pool.tile([128, TB], F16, tag="ks")
                    nc.vector.tensor_add(ksum[:], t1[:], t2[:])
                    nc.vector.tensor_copy(k8[:, ch, 0, ts], ksum[:])
                    nc.vector.tensor_sub(k8[:, ch, 1, ts],
                                         ksum[:], k8[:, ch, 0, ts])

        def emit_vproj(tt):
            """v projection for 128-key chunk tt -> vv8 (and vv16 if early)."""
            off = (tt % 4) * 128
            ts0 = (tt // 4) * TB
            pp = sc_pool.tile([128, EC], F32, tag="pp")
            if tt < 4:
                for c in range(CCH):
                    nc.tensor.matmul(
                        pp[:], x16_sb[:, c, off:off + 128],
                        wv16_sb[:, c, :], start=(c == 0), stop=(c == CCH - 1))
            else:
                for j in range(CCH // 2):
                    nc.tensor.matmul(
                        pp[:],
                        x8_sb[:, 2 * j:2 * j + 2, ts0 + off:ts0 + off + 128],
                        wv8_sb[:, 2 * j:2 * j + 2, :],
                        start=(j == 0), stop=(j == CCH // 2 - 1),
                        perf_mode=DR)
            nc.vector.tensor_copy(
                vv8[:, tt, :, 0:D], pp[:].rearrange("p (h d) -> p h d", d=D))
            if tt < 8:
                nc.vector.tensor_copy(
                    vv16[:, tt, :, 0:D],
                    pp[:].rearrange("p (h d) -> p h d", d=D))

        # --------- attention per (qb, h) ---------
        def affine_mask(e_t, i, j, a, bnd):
            # zero-fill masked region of chunk at slot i, window [a:bnd),
            # keeping (col - p - 128*j) >= 0 with col = a + pattern index
            nc.gpsimd.affine_select(
                out=e_t[:, i, a:bnd], in_=e_t[:, i, a:bnd],
                compare_op=ALU.is_ge, fill=0.0,
                base=a - 128 * j, pattern=[[1, bnd - a]],
                channel_multiplier=-1)

        def emit_attn_qb0_qkexp(h):
            po = (h % 2) * 64
            ch = h // 2
            o_ps = op_pool.tile([128, OCH, D + 1], F32, tag="o")
            e_list = []
            for pair in range(2):
                s_ps = sp_pool.tile([128, 2, TB], F32, tag="s")
                a = 256 * pair  # exp window start col
                for i in range(2):
                    j = 2 * pair + i
                    co = a  # write the full exp window (no stale PSUM)
                    nc.tensor.matmul(
                        s_ps[:, i, co:],
                        k16[po:po + D, ch, j * 128:(j + 1) * 128],
                        q16[po:po + D, ch, co:], start=True, stop=True)
                e_t = e16_pool.tile([128, 2, TB], F16, tag="e")
                nc.scalar.activation(e_t[:, :, a:], s_ps[:, :, a:],
                                     AF.Exp, scale=0.125, bias=nbias[:])
                j0 = 2 * pair
                affine_mask(e_t, 0, j0, a, a + 128)
                affine_mask(e_t, 1, j0 + 1, a, a + 256)
                e_list.append(e_t)
            return o_ps, e_list

        def emit_attn_qb0_av(h, o_ps, e_list):
            po = (h % 2) * 64
            ch = h // 2
            # st-major accumulation: a start=True matmul clears the whole
            # PSUM bank's has_written bits, so each region's group must be
            # contiguous (no other start=True to this bank in between).
            for st in range(OCH):
                for kc in range(st + 1):
                    nc.tensor.matmul(
                        o_ps[:, st, :],
                        e_list[kc // 2][:, kc % 2, st * 128:(st + 1) * 128],
                        vv16[:, kc, h, :],
                        start=(kc == 0), stop=(kc == st))
            emit_divide_transpose(0, h, po, ch, o_ps)

        def emit_attn(qb, h):
            po = (h % 2) * 64
            ch = h // 2
            o_ps = op_pool.tile([128, OCH, D + 1], F32, tag="o")
            npair = (qb + 1) * 2
            e_list = []
            for pair in range(npair):
                kc0 = 2 * pair
                diag = kc0 - 4 * qb  # >=0 for diagonal pairs (0 or 2)
                s_ps = sp_pool.tile([128, 2, TB], F32, tag="s")
                a = 128 * diag if diag > 0 else 0
                for i in range(2):
                    kc = kc0 + i
                    co = a  # write the full exp window (no stale PSUM)
                    lhsT = k8[po:po + D, ch, :, kc * 128:(kc + 1) * 128]
                    rhs = q8[po:po + D, ch,
                             qb * TB + co:(qb + 1) * TB]
                    rhs = rhs.unsqueeze(1).broadcast_to([D, 2, TB - co])
                    nc.tensor.matmul(s_ps[:, i, co:], lhsT, rhs,
                                     start=True, stop=True, perf_mode=DR)
                if qb == 1:
                    e_t = e16_pool.tile([128, 2, TB], F16, tag="e")
                else:
                    e_t = e8_pool.tile([128, 2, TB], FP8E, tag="e")
                nc.scalar.activation(e_t[:, :, a:], s_ps[:, :, a:],
                                     AF.Exp, scale=0.125, bias=nbias[:])
                if diag >= 0:
                    affine_mask(e_t, 0, diag, a, a + 128)
                    affine_mask(e_t, 1, diag + 1, a, a + 256)
                e_list.append(e_t)
            # st-major accumulation (see emit_attn_qb0 comment)
            if qb == 1:
                # fp16 AV per chunk (8 key-chunks live in vv16)
                for st in range(OCH):
                    kcs = [kc for kc in range(4 * qb + st + 1)
                           if kc < 4 * qb or (kc - 4 * qb) <= st]
                    for n_, kc in enumerate(kcs):
                        nc.tensor.matmul(
                            o_ps[:, st, :],
                            e_list[kc // 2][:, kc % 2,
                                            st * 128:(st + 1) * 128],
                            vv16[:, kc, h, :],
                            start=(n_ == 0), stop=(n_ == len(kcs) - 1))
                emit_divide_transpose(qb, h, po, ch, o_ps)
                return
            for st in range(OCH):
                last = min(npair - 1, 2 * qb + st // 2)
                for pair in range(last + 1):
                    diag = 2 * pair - 4 * qb
                    if diag > 0 and st < diag:
                        continue
                    nc.tensor.matmul(
                        o_ps[:, st, :],
                        e_list[pair][:, :, st * 128:(st + 1) * 128],
                        vv8[:, 2 * pair:2 * pair + 2, h, :],
                        start=(pair == 0), stop=(pair == last),
                        perf_mode=DR)
            emit_divide_transpose(qb, h, po, ch, o_ps)

        def emit_divide_transpose(qb, h, po, ch, o_ps):
            r = rsb_pool.tile([128, OCH], F32, tag="r")
            nc.vector.reciprocal(r[:], o_ps[:, :, D])
            if po == 0:
                o2 = osb_pool.tile([128, OCH, 128], F16, tag="o2")
                _o2_live[ch] = o2
            else:
                o2 = _o2_live[ch]
            rb = r[:].unsqueeze(2).broadcast_to([128, OCH, D])
            nc.vector.tensor_mul(o2[:, :, po:po + D], o_ps[:, :, 0:D], rb)
            if po == 64:
                for st in range(OCH):
                    nc.sync.dma_start_transpose(
                        out=oT[:, ch,
                               qb * TB + st * 128:qb * TB + (st + 1) * 128],
                        in_=o2[:, st, :])

        def emit_yproj(qb, st, eh, tail=False):
            tt = qb * OCH + st
            tsl = slice(tt * 128, (tt + 1) * 128)
            y_ps = sc_pool.tile([128, 512], F32, tag="pp")
            for c in range(OCH):
                nc.tensor.matmul(
                    y_ps[:], oT[:, c, tsl],
                    wo_sb[:, c, eh * 512:(eh + 1) * 512],
                    start=(c == 0), stop=(c == OCH - 1))
            y_sb = ysb_pool.tile([128, 512], F16, tag="ysb")
            if tail:
                nc.scalar.copy(y_sb[:], y_ps[:])
            else:
                nc.vector.tensor_copy(y_sb[:], y_ps[:])
            nc.sync.dma_start(
                out=y[tsl, eh * 512:(eh + 1) * 512], in_=y_sb[:])

        # --------- schedule ---------
        _o2_live = {}
        # Block 0: project (k,q) per chunk and immediately run the two
        # heads' QK+exp so ACT starts early; v tiles follow ch0; AV waits
        # on v naturally via deps.
        for ch in range(OCH):
            emit_qk_proj(0, "k", ch)
            emit_qk_proj(0, "q", ch)
            emit_vproj(ch)  # v key-chunks 0..3 (t-block 0)
        for h in range(HPC):
            emit_attn_qb0_av(h, *emit_attn_qb0_qkexp(h))

        # pending work queues for interleaving
        pending_proj = []
        pending_y = []

        def drain_one_proj():
            if pending_proj:
                kind, arg = pending_proj.pop(0)
                if kind == "qk":
                    emit_qk_proj(*arg)
                else:
                    emit_vproj(arg)

        pending_proj = (
            [("v", 4 + i) for i in range(4)]
            + [("qk", (1, w, ch)) for ch in range(OCH) for w in ("k", "q")]
        )
        while pending_proj:
            drain_one_proj()
        pending_y = [(0, st, eh) for st in range(OCH) for eh in range(2)]

        for qb in range(1, NTB):
            nb = qb + 1  # next block to project during this attention
            if nb < NTB:
                pending_proj = (
                    [("qk", (nb, "k", 0)), ("qk", (nb, "q", 0))]
                    + [("v", nb * 4 + i) for i in range(4)]
                    + [("qk", (nb, w, ch)) for ch in range(1, OCH)
                       for w in ("k", "q")]
                )
            for h in range(HPC):
                emit_attn(qb, h)
                # interleave next-block projections + prev-qb out-proj;
                # front-load so next block's rope lands early
                for _ in range(4 if h < 3 else 1):
                    drain_one_proj()
                if pending_y:
                    emit_yproj(*pending_y.pop(0))
            while pending_proj:
                drain_one_proj()
            while pending_y:
                emit_yproj(*pending_y.pop(0))
            pending_y = [(qb, st, eh) for st in range(OCH)
                         for eh in range(2)]
        for args in pending_y:
            emit_yproj(*args, tail=True)

        if DEBUG_DUMP:
            dumps = [("d_q16", q16, F16), ("d_k16", k16, F16),
                     ("d_q8", q8, FP8), ("d_k8", k8, FP8),
                     ("d_vv8", vv8, FP8), ("d_vv16", vv16, F16),
                     ("d_oT", oT, F16)]
            for nm, t_, dt_ in dumps:
                shp = list(t_.shape)
                dd = nc.dram_tensor(nm, shp, dt_, kind="ExternalOutput").ap()
                nc.sync.dma_start(out=dd, in_=t_[:])

    nc.compile()
    return nc


def _host_inputs(x, Wq, Wk, Wv, Wo):
    inv_freq = 1.0 / (ROPE_BASE ** (np.arange(0, D, 2, dtype=np.float64) / D))
    freqs = np.outer(np.arange(T, dtype=np.float64), inv_freq)  # [T, 32]
    emb = np.concatenate([freqs, freqs], axis=-1)               # [T, 64]
    cos1, sin1 = np.cos(emb).T, np.sin(emb).T                   # [64, T]
    cosb = np.concatenate([cos1, cos1], 0).astype(np.float16)
    sinb = np.concatenate([sin1, sin1], 0).astype(np.float16)

    # rotate-half as lhsT: out = p2t.T @ q = R2 @ q
    R = np.zeros((64, 64), dtype=np.float32)
    for i in range(32):
        R[i, i + 32] = -1.0
        R[i + 32, i] = 1.0
    R2 = np.zeros((128, 128), dtype=np.float32)
    R2[0:64, 0:64] = R
    R2[64:128, 64:128] = R
    p2t = np.ascontiguousarray(R2.T).astype(np.float16)

    f8 = ml_dtypes.float8_e4m3
    xT = [np.ascontiguousarray(x[b].T) for b in range(B)]
    x8s = [t.astype(f8) for t in xT]
    x16s = [t[:, :TB].astype(np.float16) for t in xT]
    wmaps = []
    for hg in range(2):
        cols = slice(hg * EC, (hg + 1) * EC)
        wqT = np.ascontiguousarray(Wq[cols, :].T)
        wkT = np.ascontiguousarray(Wk[cols, :].T)
        wvT = np.ascontiguousarray(Wv[cols, :].T)
        woT = np.ascontiguousarray(Wo[:, cols].T)
        wmaps.append({
            "wq8": wqT.astype(f8), "wk8": wkT.astype(f8),
            "wv8": wvT.astype(f8),
            "wq16": wqT.astype(np.float16), "wk16": wkT.astype(np.float16),
            "wv16": wvT.astype(np.float16),
            "wo16": woT.astype(np.float16),
        })
    in_maps = []
    for c in range(N_CORES):
        b, hg = c // 2, c % 2
        in_maps.append({
            "x8": x8s[b], "x16": x16s[b], "p2t": p2t,
            "cosb": cosb, "sinb": sinb,
            **wmaps[hg],
        })
    return in_maps


def kernel(x, causal_mask, Wq, Wk, Wv, Wo):
    global _NC
    x = np.asarray(x, dtype=np.float32)
    Wq = np.asarray(Wq, dtype=np.float32)
    Wk = np.asarray(Wk, dtype=np.float32)
    Wv = np.asarray(Wv, dtype=np.float32)
    Wo = np.asarray(Wo, dtype=np.float32)
    if _NC is None:
        _NC = _build()
    in_maps = _host_inputs(x, Wq, Wk, Wv, Wo)
    try:
        res = run_bass_kernel_spmd(_NC, in_maps, list(range(N_CORES)))
    except Exception:
        import time
        time.sleep(2)
        res = run_bass_kernel_spmd(_NC, in_maps, list(range(N_CORES)))
    out = np.empty((B, T, E), dtype=np.float32)
    for b in range(B):
        out[b] = (res.results[2 * b]["y"].astype(np.float32)
                  + res.results[2 * b + 1]["y"].astype(np.float32))
    return out


# revision 51
# speedup vs baseline: 1.0117x; 1.0117x over previous
"""Multi-head attention (B=4, T=2048, E=1024, H=16, D=64) on 8 TRN2 cores.

Sharding: core c handles batch b = c//2 and heads hg = c%2 (8 heads each).
No cross-device comms: each core emits a partial out-projection
y_partial[b] over its 512 head-columns; the host sums core pairs.

v2 design (per core):
  - q/k/v projections: fp8e4 DoubleRow matmuls (2 contraction chunks per
    instruction) except t-block 0 / first 512 keys which use fp16 for
    accuracy where causal attention concentrates on few keys.
  - RoPE: rotate-half via fp16 PE permutation matmul, combine on DVE.
  - QK: scoresT[k,q] per (qb, head, 128-key chunk); q-block 0 in fp16,
    later q-blocks as fp8 DoubleRow with a zeros second subtile.
  - exp on ACT, fused over chunk PAIRS (one activation per 2 chunks),
    output fp8e4 (fp16 for qb 0); causal masking via narrowed
    affine_select windows on GPSIMD.
  - AV transposed: out o[t,d] = e[k,t]^T v[k,d] with a ones column in v
    giving the softmax denominator; fp8 DoubleRow over kc pairs.
  - divide: per-partition reciprocal + broadcast multiply on DVE,
    then DMA-transpose o[t,d] -> oT[d,t] for the out-projection.
  - out-projection in fp16; y emitted fp16, host sums core pairs.
"""
import sys
import numpy as np
from contextlib import ExitStack

try:
    import concourse  # noqa: F401
except ImportError:
    sys.path.insert(0, "/opt/trn_rl_repo")

import concourse.tile as tile  # noqa: E402
from concourse import bacc, mybir  # noqa: E402
from concourse.bass_utils import run_bass_kernel_spmd  # noqa: E402
import ml_dtypes  # noqa: E402

F32 = mybir.dt.float32
F16 = mybir.dt.float16
FP8 = mybir.dt.float8e4
FP8E = mybir.dt.float8e5
AF = mybir.ActivationFunctionType
ALU = mybir.AluOpType
DR = mybir.MatmulPerfMode.DoubleRow

B, T, E, H, D = 4, 2048, 1024, 16, 64
N_CORES = 8
HPC = 8            # heads per core
EC = HPC * D       # 512 head-columns per core
TB = 512           # t/q block
KC = 128           # k chunk
NTB = T // TB      # 4
NTT = T // KC      # 16
CCH = E // 128     # 8 contraction chunks for x projections
OCH = EC // 128    # 4 chunks of the per-core head-column dim
ROPE_BASE = 10000.0

_NC = None
DEBUG_DUMP = False


def _build():
    nc = bacc.Bacc("TRN2", target_bir_lowering=False, debug=False,
                   num_devices=N_CORES)
    ap = {}

    def din(name, shape, dt):
        ap[name] = nc.dram_tensor(name, shape, dt, kind="ExternalInput").ap()

    din("x8", [E, T], FP8)            # x[b].T fp8
    din("x16", [E, TB], F16)          # x[b].T[:, :512] fp16
    din("wq8", [E, EC], FP8)
    din("wk8", [E, EC], FP8)
    din("wv8", [E, EC], FP8)
    din("wq16", [E, EC], F16)
    din("wk16", [E, EC], F16)
    din("wv16", [E, EC], F16)
    din("wo16", [EC, E], F16)
    din("p2t", [128, 128], F16)       # rotate-half permutation (lhsT form)
    din("cosb", [128, T], F16)        # cos dup'd over 2 heads, [2*64, T]
    din("sinb", [128, T], F16)
    y = nc.dram_tensor("y", [T, E], F16, kind="ExternalOutput").ap()

    with tile.TileContext(nc) as tc, ExitStack() as octx:
        persist = octx.enter_context(tc.tile_pool(name="persist", bufs=1))
        x8_sb = persist.tile([128, CCH, T], FP8, tag="x8")
        x16_sb = persist.tile([128, CCH, TB], F16, tag="x16")
        wq8_sb = persist.tile([128, CCH, EC], FP8, tag="wq8")
        wk8_sb = persist.tile([128, CCH, EC], FP8, tag="wk8")
        wv8_sb = persist.tile([128, CCH, EC], FP8, tag="wv8")
        wq16_sb = persist.tile([128, CCH, EC], F16, tag="wq16")
        wk16_sb = persist.tile([128, CCH, EC], F16, tag="wk16")
        wv16_sb = persist.tile([128, CCH, EC], F16, tag="wv16")
        wo_sb = persist.tile([128, OCH, E], F16, tag="wo")
        p2t_sb = persist.tile([128, 128], F16, tag="p2t")
        cos_sb = persist.tile([128, T], F16, tag="cos")
        sin_sb = persist.tile([128, T], F16, tag="sin")
        q16 = persist.tile([128, OCH, TB], F16, tag="q16")
        k16 = persist.tile([128, OCH, TB], F16, tag="k16")
        q8 = persist.tile([128, OCH, T], FP8, tag="q8")
        k8 = persist.tile([128, OCH, 2, T], FP8, tag="k8")  # slot1 = zeros
        vv8 = persist.tile([128, NTT, HPC, D + 1], FP8, tag="vv8")
        vv16 = persist.tile([128, 2 * NTB, HPC, D + 1], F16, tag="vv16")
        oT = persist.tile([128, OCH, T], F16, tag="oT")

        # --------- initial DMAs (startup-critical first) ---------
        def load(dst, src, n=1):
            # split along dim0 chunks of dst's second axis for pipelining
            nc.sync.dma_start(out=dst, in_=src)

        # two HWDGE queues: SP carries x16/wq16/... , ACT carries wk16 and
        # the small rope constants (ACT is idle until the first exp).
        x16r = ap["x16"].rearrange("(c p) t -> p c t", p=128)
        x8r = ap["x8"].rearrange("(c p) t -> p c t", p=128)
        wk16src = ap["wk16"].rearrange("(c p) e -> p c e", p=128)
        nc.scalar.dma_start(out=p2t_sb, in_=ap["p2t"])
        nc.scalar.dma_start(out=cos_sb[:, 0:TB], in_=ap["cosb"][:, 0:TB])
        nc.scalar.dma_start(out=sin_sb[:, 0:TB], in_=ap["sinb"][:, 0:TB])
        for m in range(OCH):
            nc.scalar.dma_start(
                out=wk16_sb[:, :, m * 128:(m + 1) * 128],
                in_=wk16src[:, :, m * 128:(m + 1) * 128])
        nc.scalar.dma_start(out=cos_sb[:, TB:], in_=ap["cosb"][:, TB:])
        nc.scalar.dma_start(out=sin_sb[:, TB:], in_=ap["sinb"][:, TB:])
        nc.sync.dma_start(out=x16_sb, in_=x16r)
        wq16src = ap["wq16"].rearrange("(c p) e -> p c e", p=128)
        for m in range(OCH):
            nc.sync.dma_start(out=wq16_sb[:, :, m * 128:(m + 1) * 128],
                              in_=wq16src[:, :, m * 128:(m + 1) * 128])
        nc.sync.dma_start(
            out=wv16_sb, in_=ap["wv16"].rearrange("(c p) e -> p c e", p=128))
        nc.sync.dma_start(out=x8_sb, in_=x8r)
        for name, dst in (("wk8", wk8_sb), ("wq8", wq8_sb), ("wv8", wv8_sb)):
            nc.sync.dma_start(
                out=dst, in_=ap[name].rearrange("(c p) e -> p c e", p=128))
        nc.sync.dma_start(
            out=wo_sb, in_=ap["wo16"].rearrange("(c p) e -> p c e", p=128))

        # zeros + ones init (Pool; overlaps first projections)
        nc.gpsimd.memset(vv8[:, :, :, D], 1.0)
        nc.gpsimd.memset(vv16[:, :, :, D], 1.0)
        # exp bias: e^(s/8 - 2) keeps fp8 e4m3 in range for extreme scores
        nbias = persist.tile([128, 1], F32, tag="nbias")
        nc.vector.memset(nbias[:], -2.0)

        # --------- pools ---------
        sp_pool = octx.enter_context(
            tc.tile_pool(name="sp", bufs=2, space="PSUM"))    # 4 banks
        op_pool = octx.enter_context(
            tc.tile_pool(name="op", bufs=2, space="PSUM"))    # 2 banks
        sc_pool = octx.enter_context(
            tc.tile_pool(name="sc", bufs=2, space="PSUM"))    # 2 banks
        e8_pool = octx.enter_context(tc.tile_pool(name="e8", bufs=12))
        e16_pool = octx.enter_context(tc.tile_pool(name="e16", bufs=10))
        tmp_pool = octx.enter_context(tc.tile_pool(name="tmp", bufs=6))
        raw_pool = octx.enter_context(tc.tile_pool(name="raw", bufs=4))
        osb_pool = octx.enter_context(tc.tile_pool(name="osb", bufs=4))
        ysb_pool = octx.enter_context(tc.tile_pool(name="ysb", bufs=4))
        rsb_pool = octx.enter_context(tc.tile_pool(name="rsb", bufs=4))

        # --------- projection emitters ---------
        def emit_qk_proj(b, which, ch):
            """Project + rope q or k for t-block b, head-pair chunk ch."""
            ts = slice(b * TB, (b + 1) * TB)
            pp = sc_pool.tile([128, TB], F32, tag="pp")
            if b == 0:
                w = wq16_sb if which == "q" else wk16_sb
                for c in range(CCH):
                    nc.tensor.matmul(
                        pp[:], w[:, c, ch * 128:(ch + 1) * 128],
                        x16_sb[:, c, :], start=(c == 0), stop=(c == CCH - 1))
            else:
                w = wq8_sb if which == "q" else wk8_sb
                for j in range(CCH // 2):
                    nc.tensor.matmul(
                        pp[:], w[:, 2 * j:2 * j + 2, ch * 128:(ch + 1) * 128],
                        x8_sb[:, 2 * j:2 * j + 2, ts],
                        start=(j == 0), stop=(j == CCH // 2 - 1),
                        perf_mode=DR)
            raw = raw_pool.tile([128, TB], F16, tag="raw")
            if b == 0 and ch == 0:
                nc.scalar.copy(raw[:], pp[:])
            else:
                nc.vector.tensor_copy(raw[:], pp[:])
            sw = sc_pool.tile([128, TB], F32, tag="pp")
            nc.tensor.matmul(sw[:], p2t_sb[:], raw[:], start=True, stop=True)
            t1 = tmp_pool.tile([128, TB], F16, tag="t1")
            nc.vector.tensor_mul(t1[:], raw[:], cos_sb[:, ts])
            t2 = tmp_pool.tile([128, TB], F16, tag="t2")
            nc.vector.tensor_mul(t2[:], sw[:], sin_sb[:, ts])
            if b == 0:
                dst16 = q16 if which == "q" else k16
                nc.vector.tensor_add(dst16[:, ch, :], t1[:], t2[:])
                if which == "k":
                    nc.vector.tensor_add(k8[:, ch, 0, 0:TB], t1[:], t2[:])
                    # slot1 = fp16 k minus fp8 k: QK DR adds k_res @ q,
                    # cancelling the k-side quantization error
                    nc.vector.tensor_sub(k8[:, ch, 1, 0:TB],
                                         dst16[:, ch, :], k8[:, ch, 0, 0:TB])
            else:
                if which == "q":
                    nc.vector.tensor_add(q8[:, ch, ts], t1[:], t2[:])
                else:
                    ksum = tmp_pool.tile([128, TB], F16, tag="ks")
                    nc.vector.tensor_add(ksum[:], t1[:], t2[:])
                    nc.vector.tensor_copy(k8[:, ch, 0, ts], ksum[:])
                    nc.vector.tensor_sub(k8[:, ch, 1, ts],
                                         ksum[:], k8[:, ch, 0, ts])

        def emit_vproj(tt):
            """v projection for 128-key chunk tt -> vv8 (and vv16 if early)."""
            off = (tt % 4) * 128
            ts0 = (tt // 4) * TB
            pp = sc_pool.tile([128, EC], F32, tag="pp")
            if tt < 4:
                for c in range(CCH):
                    nc.tensor.matmul(
                        pp[:], x16_sb[:, c, off:off + 128],
                        wv16_sb[:, c, :], start=(c == 0), stop=(c == CCH - 1))
            else:
                for j in range(CCH // 2):
                    nc.tensor.matmul(
                        pp[:],
                        x8_sb[:, 2 * j:2 * j + 2, ts0 + off:ts0 + off + 128],
                        wv8_sb[:, 2 * j:2 * j + 2, :],
                        start=(j == 0), stop=(j == CCH // 2 - 1),
                        perf_mode=DR)
            nc.vector.tensor_copy(
                vv8[:, tt, :, 0:D], pp[:].rearrange("p (h d) -> p h d", d=D))
            if tt < 8:
                nc.vector.tensor_copy(
                    vv16[:, tt, :, 0:D],
                    pp[:].rearrange("p (h d) -> p h d", d=D))

        # --------- attention per (qb, h) ---------
        def affine_mask(e_t, i, j, a, bnd):
            # zero-fill masked region of chunk at slot i, window [a:bnd),
            # keeping (col - p - 128*j) >= 0 with col = a + pattern index
            nc.gpsimd.affine_select(
                out=e_t[:, i, a:bnd], in_=e_t[:, i, a:bnd],
                compare_op=ALU.is_ge, fill=0.0,
                base=a - 128 * j, pattern=[[1, bnd - a]],
                channel_multiplier=-1)

        def emit_attn_qb0_qkexp(h):
            po = (h % 2) * 64
            ch = h // 2
            o_ps = op_pool.tile([128, OCH, D + 1], F32, tag="o")
            e_list = []
            for pair in range(2):
                s_ps = sp_pool.tile([128, 2, TB], F32, tag="s")
                a = 256 * pair  # exp window start col
                for i in range(2):
                    j = 2 * pair + i
                    co = a  # write the full exp window (no stale PSUM)
                    nc.tensor.matmul(
                        s_ps[:, i, co:],
                        k16[po:po + D, ch, j * 128:(j + 1) * 128],
                        q16[po:po + D, ch, co:], start=True, stop=True)
                e_t = e16_pool.tile([128, 2, TB], F16, tag="e")
                nc.scalar.activation(e_t[:, :, a:], s_ps[:, :, a:],
                                     AF.Exp, scale=0.125, bias=nbias[:])
                j0 = 2 * pair
                affine_mask(e_t, 0, j0, a, a + 128)
                affine_mask(e_t, 1, j0 + 1, a, a + 256)
                e_list.append(e_t)
            return o_ps, e_list

        def emit_attn_qb0_av(h, o_ps, e_list):
            po = (h % 2) * 64
            ch = h // 2
            # st-major accumulation: a start=True matmul clears the whole
            # PSUM bank's has_written bits, so each region's group must be
            # contiguous (no other start=True to this bank in between).
            for st in range(OCH):
                for kc in range(st + 1):
                    nc.tensor.matmul(
                        o_ps[:, st, :],
                        e_list[kc // 2][:, kc % 2, st * 128:(st + 1) * 128],
                        vv16[:, kc, h, :],
                        start=(kc == 0), stop=(kc == st))
            emit_divide_transpose(0, h, po, ch, o_ps)

        def emit_attn(qb, h):
            po = (h % 2) * 64
            ch = h // 2
            o_ps = op_pool.tile([128, OCH, D + 1], F32, tag="o")
            npair = (qb + 1) * 2
            e_list = []
            for pair in range(npair):
                kc0 = 2 * pair
                diag = kc0 - 4 * qb  # >=0 for diagonal pairs (0 or 2)
                s_ps = sp_pool.tile([128, 2, TB], F32, tag="s")
                a = 128 * diag if diag > 0 else 0
                for i in range(2):
                    kc = kc0 + i
                    co = a  # write the full exp window (no stale PSUM)
                    lhsT = k8[po:po + D, ch, :, kc * 128:(kc + 1) * 128]
                    rhs = q8[po:po + D, ch,
                             qb * TB + co:(qb + 1) * TB]
                    rhs = rhs.unsqueeze(1).broadcast_to([D, 2, TB - co])
                    nc.tensor.matmul(s_ps[:, i, co:], lhsT, rhs,
                                     start=True, stop=True, perf_mode=DR)
                if qb == 1:
                    e_t = e16_pool.tile([128, 2, TB], F16, tag="e")
                else:
                    e_t = e8_pool.tile([128, 2, TB], FP8E, tag="e")
                nc.scalar.activation(e_t[:, :, a:], s_ps[:, :, a:],
                                     AF.Exp, scale=0.125, bias=nbias[:])
                if diag >= 0:
                    affine_mask(e_t, 0, diag, a, a + 128)
                    affine_mask(e_t, 1, diag + 1, a, a + 256)
                e_list.append(e_t)
            # st-major accumulation (see emit_attn_qb0 comment)
            if qb == 1:
                # fp16 AV per chunk (8 key-chunks live in vv16)
                for st in range(OCH):
                    kcs = [kc for kc in range(4 * qb + st + 1)
                           if kc < 4 * qb or (kc - 4 * qb) <= st]
                    for n_, kc in enumerate(kcs):
                        nc.tensor.matmul(
                            o_ps[:, st, :],
                            e_list[kc // 2][:, kc % 2,
                                            st * 128:(st + 1) * 128],
                            vv16[:, kc, h, :],
                            start=(n_ == 0), stop=(n_ == len(kcs) - 1))
                emit_divide_transpose(qb, h, po, ch, o_ps)
                return
            for st in range(OCH):
                last = min(npair - 1, 2 * qb + st // 2)
                for pair in range(last + 1):
                    diag = 2 * pair - 4 * qb
                    if diag > 0 and st < diag:
                        continue
                    nc.tensor.matmul(
                        o_ps[:, st, :],
                        e_list[pair][:, :, st * 128:(st + 1) * 128],
                        vv8[:, 2 * pair:2 * pair + 2, h, :],
                        start=(pair == 0), stop=(pair == last),
                        perf_mode=DR)
            emit_divide_transpose(qb, h, po, ch, o_ps)

        def emit_divide_transpose(qb, h, po, ch, o_ps):
            r = rsb_pool.tile([128, OCH], F32, tag="r")
            nc.vector.reciprocal(r[:], o_ps[:, :, D])
            if po == 0:
                o2 = osb_pool.tile([128, OCH, 128], F16, tag="o2")
                _o2_live[ch] = o2
            else:
                o2 = _o2_live[ch]
            rb = r[:].unsqueeze(2).broadcast_to([128, OCH, D])
            nc.vector.tensor_mul(o2[:, :, po:po + D], o_ps[:, :, 0:D], rb)
            if po == 64:
                for st in range(OCH):
                    nc.sync.dma_start_transpose(
                        out=oT[:, ch,
                               qb * TB + st * 128:qb * TB + (st + 1) * 128],
                        in_=o2[:, st, :])

        def emit_yproj(qb, st, eh, tail=False):
            tt = qb * OCH + st
            tsl = slice(tt * 128, (tt + 1) * 128)
            y_ps = sc_pool.tile([128, 512], F32, tag="pp")
            for c in range(OCH):
                nc.tensor.matmul(
                    y_ps[:], oT[:, c, tsl],
                    wo_sb[:, c, eh * 512:(eh + 1) * 512],
                    start=(c == 0), stop=(c == OCH - 1))
            y_sb = ysb_pool.tile([128, 512], F16, tag="ysb")
            if tail:
                nc.scalar.copy(y_sb[:], y_ps[:])
            else:
                nc.vector.tensor_copy(y_sb[:], y_ps[:])
            nc.sync.dma_start(
                out=y[tsl, eh * 512:(eh + 1) * 512], in_=y_sb[:])

        # --------- schedule ---------
        _o2_live = {}
        # Block 0: project (k,q) per chunk and immediately run the two
        # heads' QK+exp so ACT starts early; v tiles follow ch0; AV waits
        # on v naturally via deps.
        for ch in range(OCH):
            emit_qk_proj(0, "k", ch)
            emit_qk_proj(0, "q", ch)
            emit_vproj(ch)  # v key-chunks 0..3 (t-block 0)
        for h in range(HPC):
            emit_attn_qb0_av(h, *emit_attn_qb0_qkexp(h))

        # pending work queues for interleaving
        pending_proj = []
        pending_y = []

        def drain_one_proj():
            if pending_proj:
                kind, arg = pending_proj.pop(0)
                if kind == "qk":
                    emit_qk_proj(*arg)
                else:
                    emit_vproj(arg)

        pending_proj = (
            [("v", 4 + i) for i in range(4)]
            + [("qk", (1, w, ch)) for ch in range(OCH) for w in ("k", "q")]
        )
        while pending_proj:
            drain_one_proj()
        pending_y = [(0, st, eh) for st in range(OCH) for eh in range(2)]

        for qb in range(1, NTB):
            nb = qb + 1  # next block to project during this attention
            if nb < NTB:
                pending_proj = (
                    [("qk", (nb, "k", 0)), ("qk", (nb, "q", 0))]
                    + [("v", nb * 4 + i) for i in range(4)]
                    + [("qk", (nb, w, ch)) for ch in range(1, OCH)
                       for w in ("k", "q")]
                )
            for h in range(HPC):
                emit_attn(qb, h)
                # interleave next-block projections + prev-qb out-proj;
                # front-load so next block's rope lands early
                for _ in range(4 if h < 3 else 1):
                    drain_one_proj()
                if pending_y:
                    emit_yproj(*pending_y.pop(0))
            while pending_proj:
                drain_one_proj()
            while pending_y:
                emit_yproj(*pending_y.pop(0))
            pending_y = [(qb, st, eh) for st in range(OCH)
                         for eh in range(2)]
        for args in pending_y:
            emit_yproj(*args, tail=True)

        if DEBUG_DUMP:
            dumps = [("d_q16", q16, F16), ("d_k16", k16, F16),
                     ("d_q8", q8, FP8), ("d_k8", k8, FP8),
                     ("d_vv8", vv8, FP8), ("d_vv16", vv16, F16),
                     ("d_oT", oT, F16)]
            for nm, t_, dt_ in dumps:
                shp = list(t_.shape)
                dd = nc.dram_tensor(nm, shp, dt_, kind="ExternalOutput").ap()
                nc.sync.dma_start(out=dd, in_=t_[:])

    nc.compile()
    return nc


def _host_inputs(x, Wq, Wk, Wv, Wo):
    inv_freq = 1.0 / (ROPE_BASE ** (np.arange(0, D, 2, dtype=np.float64) / D))
    freqs = np.outer(np.arange(T, dtype=np.float64), inv_freq)  # [T, 32]
    emb = np.concatenate([freqs, freqs], axis=-1)               # [T, 64]
    cos1, sin1 = np.cos(emb).T, np.sin(emb).T                   # [64, T]
    cosb = np.concatenate([cos1, cos1], 0).astype(np.float16)
    sinb = np.concatenate([sin1, sin1], 0).astype(np.float16)

    # rotate-half as lhsT: out = p2t.T @ q = R2 @ q
    R = np.zeros((64, 64), dtype=np.float32)
    for i in range(32):
        R[i, i + 32] = -1.0
        R[i + 32, i] = 1.0
    R2 = np.zeros((128, 128), dtype=np.float32)
    R2[0:64, 0:64] = R
    R2[64:128, 64:128] = R
    p2t = np.ascontiguousarray(R2.T).astype(np.float16)

    f8 = ml_dtypes.float8_e4m3
    xT = [np.ascontiguousarray(x[b].T) for b in range(B)]
    x8s = [t.astype(f8) for t in xT]
    x16s = [t[:, :TB].astype(np.float16) for t in xT]
    wmaps = []
    for hg in range(2):
        cols = slice(hg * EC, (hg + 1) * EC)
        wqT = np.ascontiguousarray(Wq[cols, :].T)
        wkT = np.ascontiguousarray(Wk[cols, :].T)
        wvT = np.ascontiguousarray(Wv[cols, :].T)
        woT = np.ascontiguousarray(Wo[:, cols].T)
        wmaps.append({
            "wq8": wqT.astype(f8), "wk8": wkT.astype(f8),
            "wv8": wvT.astype(f8),
            "wq16": wqT.astype(np.float16), "wk16": wkT.astype(np.float16),
            "wv16": wvT.astype(np.float16),
            "wo16": woT.astype(np.float16),
        })
    in_maps = []
    for c in range(N_CORES):
        b, hg = c // 2, c % 2
        in_maps.append({
            "x8": x8s[b], "x16": x16s[b], "p2t": p2t,
            "cosb": cosb, "sinb": sinb,
            **wmaps[hg],
        })
    return in_maps


def kernel(x, causal_mask, Wq, Wk, Wv, Wo):
    global _NC
    x = np.asarray(x, dtype=np.float32)
    Wq = np.asarray(Wq, dtype=np.float32)
    Wk = np.asarray(Wk, dtype=np.float32)
    Wv = np.asarray(Wv, dtype=np.float32)
    Wo = np.asarray(Wo, dtype=np.float32)
    if _NC is None:
        _NC = _build()
    in_maps = _host_inputs(x, Wq, Wk, Wv, Wo)
    try:
        res = run_bass_kernel_spmd(_NC, in_maps, list(range(N_CORES)))
    except Exception:
        import time
        time.sleep(2)
        res = run_bass_kernel_spmd(_NC, in_maps, list(range(N_CORES)))
    out = np.empty((B, T, E), dtype=np.float32)
    for b in range(B):
        out[b] = (res.results[2 * b]["y"].astype(np.float32)
                  + res.results[2 * b + 1]["y"].astype(np.float32))
    return out


# revision 52
# speedup vs baseline: 1.0125x; 1.0008x over previous
"""Multi-head attention (B=4, T=2048, E=1024, H=16, D=64) on 8 TRN2 cores.

Sharding: core c handles batch b = c//2 and heads hg = c%2 (8 heads each).
No cross-device comms: each core emits a partial out-projection
y_partial[b] over its 512 head-columns; the host sums core pairs.

v2 design (per core):
  - q/k/v projections: fp8e4 DoubleRow matmuls (2 contraction chunks per
    instruction) except t-block 0 / first 512 keys which use fp16 for
    accuracy where causal attention concentrates on few keys.
  - RoPE: rotate-half via fp16 PE permutation matmul, combine on DVE.
  - QK: scoresT[k,q] per (qb, head, 128-key chunk); q-block 0 in fp16,
    later q-blocks as fp8 DoubleRow with a zeros second subtile.
  - exp on ACT, fused over chunk PAIRS (one activation per 2 chunks),
    output fp8e4 (fp16 for qb 0); causal masking via narrowed
    affine_select windows on GPSIMD.
  - AV transposed: out o[t,d] = e[k,t]^T v[k,d] with a ones column in v
    giving the softmax denominator; fp8 DoubleRow over kc pairs.
  - divide: per-partition reciprocal + broadcast multiply on DVE,
    then DMA-transpose o[t,d] -> oT[d,t] for the out-projection.
  - out-projection in fp16; y emitted fp16, host sums core pairs.
"""
import sys
import numpy as np
from contextlib import ExitStack

try:
    import concourse  # noqa: F401
except ImportError:
    sys.path.insert(0, "/opt/trn_rl_repo")

import concourse.tile as tile  # noqa: E402
from concourse import bacc, mybir  # noqa: E402
from concourse.bass_utils import run_bass_kernel_spmd  # noqa: E402
import ml_dtypes  # noqa: E402

F32 = mybir.dt.float32
F16 = mybir.dt.float16
FP8 = mybir.dt.float8e4
FP8E = mybir.dt.float8e5
AF = mybir.ActivationFunctionType
ALU = mybir.AluOpType
DR = mybir.MatmulPerfMode.DoubleRow

B, T, E, H, D = 4, 2048, 1024, 16, 64
N_CORES = 8
HPC = 8            # heads per core
EC = HPC * D       # 512 head-columns per core
TB = 512           # t/q block
KC = 128           # k chunk
NTB = T // TB      # 4
NTT = T // KC      # 16
CCH = E // 128     # 8 contraction chunks for x projections
OCH = EC // 128    # 4 chunks of the per-core head-column dim
ROPE_BASE = 10000.0

_NC = None
DEBUG_DUMP = False


def _build():
    nc = bacc.Bacc("TRN2", target_bir_lowering=False, debug=False,
                   num_devices=N_CORES)
    ap = {}

    def din(name, shape, dt):
        ap[name] = nc.dram_tensor(name, shape, dt, kind="ExternalInput").ap()

    din("x8", [E, T], FP8)            # x[b].T fp8
    din("x16", [E, TB], F16)          # x[b].T[:, :512] fp16
    din("wq8", [E, EC], FP8)
    din("wk8", [E, EC], FP8)
    din("wv8", [E, EC], FP8)
    din("wq16", [E, EC], F16)
    din("wk16", [E, EC], F16)
    din("wv16", [E, EC], F16)
    din("wo16", [EC, E], F16)
    din("p2t", [128, 128], F16)       # rotate-half permutation (lhsT form)
    din("cosb", [128, T], F16)        # cos dup'd over 2 heads, [2*64, T]
    din("sinb", [128, T], F16)
    y = nc.dram_tensor("y", [T, E], F16, kind="ExternalOutput").ap()

    with tile.TileContext(nc) as tc, ExitStack() as octx:
        persist = octx.enter_context(tc.tile_pool(name="persist", bufs=1))
        x8_sb = persist.tile([128, CCH, T], FP8, tag="x8")
        x16_sb = persist.tile([128, CCH, TB], F16, tag="x16")
        wq8_sb = persist.tile([128, CCH, EC], FP8, tag="wq8")
        wk8_sb = persist.tile([128, CCH, EC], FP8, tag="wk8")
        wv8_sb = persist.tile([128, CCH, EC], FP8, tag="wv8")
        wq16_sb = persist.tile([128, CCH, EC], F16, tag="wq16")
        wk16_sb = persist.tile([128, CCH, EC], F16, tag="wk16")
        wv16_sb = persist.tile([128, CCH, EC], F16, tag="wv16")
        wo_sb = persist.tile([128, OCH, E], F16, tag="wo")
        p2t_sb = persist.tile([128, 128], F16, tag="p2t")
        cos_sb = persist.tile([128, T], F16, tag="cos")
        sin_sb = persist.tile([128, T], F16, tag="sin")
        q16 = persist.tile([128, OCH, TB], F16, tag="q16")
        k16 = persist.tile([128, OCH, TB], F16, tag="k16")
        q8 = persist.tile([128, OCH, T], FP8, tag="q8")
        k8 = persist.tile([128, OCH, 2, T], FP8, tag="k8")  # slot1 = zeros
        vv8 = persist.tile([128, NTT, HPC, D + 1], FP8, tag="vv8")
        vv16 = persist.tile([128, 2 * NTB, HPC, D + 1], F16, tag="vv16")
        oT = persist.tile([128, OCH, T], F16, tag="oT")

        # --------- initial DMAs (startup-critical first) ---------
        def load(dst, src, n=1):
            # split along dim0 chunks of dst's second axis for pipelining
            nc.sync.dma_start(out=dst, in_=src)

        # two HWDGE queues: SP carries x16/wq16/... , ACT carries wk16 and
        # the small rope constants (ACT is idle until the first exp).
        x16r = ap["x16"].rearrange("(c p) t -> p c t", p=128)
        x8r = ap["x8"].rearrange("(c p) t -> p c t", p=128)
        wk16src = ap["wk16"].rearrange("(c p) e -> p c e", p=128)
        nc.scalar.dma_start(out=p2t_sb, in_=ap["p2t"])
        nc.scalar.dma_start(out=cos_sb[:, 0:TB], in_=ap["cosb"][:, 0:TB])
        nc.scalar.dma_start(out=sin_sb[:, 0:TB], in_=ap["sinb"][:, 0:TB])
        for m in range(OCH):
            nc.scalar.dma_start(
                out=wk16_sb[:, :, m * 128:(m + 1) * 128],
                in_=wk16src[:, :, m * 128:(m + 1) * 128])
        nc.scalar.dma_start(out=cos_sb[:, TB:], in_=ap["cosb"][:, TB:])
        nc.scalar.dma_start(out=sin_sb[:, TB:], in_=ap["sinb"][:, TB:])
        nc.sync.dma_start(out=x16_sb, in_=x16r)
        wq16src = ap["wq16"].rearrange("(c p) e -> p c e", p=128)
        for m in range(OCH):
            nc.sync.dma_start(out=wq16_sb[:, :, m * 128:(m + 1) * 128],
                              in_=wq16src[:, :, m * 128:(m + 1) * 128])
        nc.sync.dma_start(
            out=wv16_sb, in_=ap["wv16"].rearrange("(c p) e -> p c e", p=128))
        nc.sync.dma_start(out=x8_sb, in_=x8r)
        for name, dst in (("wk8", wk8_sb), ("wq8", wq8_sb), ("wv8", wv8_sb)):
            nc.sync.dma_start(
                out=dst, in_=ap[name].rearrange("(c p) e -> p c e", p=128))
        nc.sync.dma_start(
            out=wo_sb, in_=ap["wo16"].rearrange("(c p) e -> p c e", p=128))

        # zeros + ones init (Pool; overlaps first projections)
        nc.gpsimd.memset(vv8[:, :, :, D], 1.0)
        nc.gpsimd.memset(vv16[:, :, :, D], 1.0)
        # exp bias: e^(s/8 - 2) keeps fp8 e4m3 in range for extreme scores
        nbias = persist.tile([128, 1], F32, tag="nbias")
        nc.vector.memset(nbias[:], -2.0)

        # --------- pools ---------
        sp_pool = octx.enter_context(
            tc.tile_pool(name="sp", bufs=2, space="PSUM"))    # 4 banks
        op_pool = octx.enter_context(
            tc.tile_pool(name="op", bufs=2, space="PSUM"))    # 2 banks
        sc_pool = octx.enter_context(
            tc.tile_pool(name="sc", bufs=2, space="PSUM"))    # 2 banks
        e8_pool = octx.enter_context(tc.tile_pool(name="e8", bufs=13))
        e16_pool = octx.enter_context(tc.tile_pool(name="e16", bufs=10))
        tmp_pool = octx.enter_context(tc.tile_pool(name="tmp", bufs=6))
        raw_pool = octx.enter_context(tc.tile_pool(name="raw", bufs=4))
        osb_pool = octx.enter_context(tc.tile_pool(name="osb", bufs=4))
        ysb_pool = octx.enter_context(tc.tile_pool(name="ysb", bufs=4))
        rsb_pool = octx.enter_context(tc.tile_pool(name="rsb", bufs=4))

        # --------- projection emitters ---------
        def emit_qk_proj(b, which, ch):
            """Project + rope q or k for t-block b, head-pair chunk ch."""
            ts = slice(b * TB, (b + 1) * TB)
            pp = sc_pool.tile([128, TB], F32, tag="pp")
            if b == 0:
                w = wq16_sb if which == "q" else wk16_sb
                for c in range(CCH):
                    nc.tensor.matmul(
                        pp[:], w[:, c, ch * 128:(ch + 1) * 128],
                        x16_sb[:, c, :], start=(c == 0), stop=(c == CCH - 1))
            else:
                w = wq8_sb if which == "q" else wk8_sb
                for j in range(CCH // 2):
                    nc.tensor.matmul(
                        pp[:], w[:, 2 * j:2 * j + 2, ch * 128:(ch + 1) * 128],
                        x8_sb[:, 2 * j:2 * j + 2, ts],
                        start=(j == 0), stop=(j == CCH // 2 - 1),
                        perf_mode=DR)
            raw = raw_pool.tile([128, TB], F16, tag="raw")
            if b == 0 and ch == 0:
                nc.scalar.copy(raw[:], pp[:])
            else:
                nc.vector.tensor_copy(raw[:], pp[:])
            sw = sc_pool.tile([128, TB], F32, tag="pp")
            nc.tensor.matmul(sw[:], p2t_sb[:], raw[:], start=True, stop=True)
            t1 = tmp_pool.tile([128, TB], F16, tag="t1")
            nc.vector.tensor_mul(t1[:], raw[:], cos_sb[:, ts])
            t2 = tmp_pool.tile([128, TB], F16, tag="t2")
            nc.vector.tensor_mul(t2[:], sw[:], sin_sb[:, ts])
            if b == 0:
                dst16 = q16 if which == "q" else k16
                nc.vector.tensor_add(dst16[:, ch, :], t1[:], t2[:])
                if which == "k":
                    nc.vector.tensor_add(k8[:, ch, 0, 0:TB], t1[:], t2[:])
                    # slot1 = fp16 k minus fp8 k: QK DR adds k_res @ q,
                    # cancelling the k-side quantization error
                    nc.vector.tensor_sub(k8[:, ch, 1, 0:TB],
                                         dst16[:, ch, :], k8[:, ch, 0, 0:TB])
            else:
                if which == "q":
                    nc.vector.tensor_add(q8[:, ch, ts], t1[:], t2[:])
                else:
                    ksum = tmp_pool.tile([128, TB], F16, tag="ks")
                    nc.vector.tensor_add(ksum[:], t1[:], t2[:])
                    nc.vector.tensor_copy(k8[:, ch, 0, ts], ksum[:])
                    nc.vector.tensor_sub(k8[:, ch, 1, ts],
                                         ksum[:], k8[:, ch, 0, ts])

        def emit_vproj(tt):
            """v projection for 128-key chunk tt -> vv8 (and vv16 if early)."""
            off = (tt % 4) * 128
            ts0 = (tt // 4) * TB
            pp = sc_pool.tile([128, EC], F32, tag="pp")
            if tt < 4:
                for c in range(CCH):
                    nc.tensor.matmul(
                        pp[:], x16_sb[:, c, off:off + 128],
                        wv16_sb[:, c, :], start=(c == 0), stop=(c == CCH - 1))
            else:
                for j in range(CCH // 2):
                    nc.tensor.matmul(
                        pp[:],
                        x8_sb[:, 2 * j:2 * j + 2, ts0 + off:ts0 + off + 128],
                        wv8_sb[:, 2 * j:2 * j + 2, :],
                        start=(j == 0), stop=(j == CCH // 2 - 1),
                        perf_mode=DR)
            nc.vector.tensor_copy(
                vv8[:, tt, :, 0:D], pp[:].rearrange("p (h d) -> p h d", d=D))
            if tt < 8:
                nc.vector.tensor_copy(
                    vv16[:, tt, :, 0:D],
                    pp[:].rearrange("p (h d) -> p h d", d=D))

        # --------- attention per (qb, h) ---------
        def affine_mask(e_t, i, j, a, bnd):
            # zero-fill masked region of chunk at slot i, window [a:bnd),
            # keeping (col - p - 128*j) >= 0 with col = a + pattern index
            nc.gpsimd.affine_select(
                out=e_t[:, i, a:bnd], in_=e_t[:, i, a:bnd],
                compare_op=ALU.is_ge, fill=0.0,
                base=a - 128 * j, pattern=[[1, bnd - a]],
                channel_multiplier=-1)

        def emit_attn_qb0_qkexp(h):
            po = (h % 2) * 64
            ch = h // 2
            o_ps = op_pool.tile([128, OCH, D + 1], F32, tag="o")
            e_list = []
            for pair in range(2):
                s_ps = sp_pool.tile([128, 2, TB], F32, tag="s")
                a = 256 * pair  # exp window start col
                for i in range(2):
                    j = 2 * pair + i
                    co = a  # write the full exp window (no stale PSUM)
                    nc.tensor.matmul(
                        s_ps[:, i, co:],
                        k16[po:po + D, ch, j * 128:(j + 1) * 128],
                        q16[po:po + D, ch, co:], start=True, stop=True)
                e_t = e16_pool.tile([128, 2, TB], F16, tag="e")
                nc.scalar.activation(e_t[:, :, a:], s_ps[:, :, a:],
                                     AF.Exp, scale=0.125, bias=nbias[:])
                j0 = 2 * pair
                affine_mask(e_t, 0, j0, a, a + 128)
                affine_mask(e_t, 1, j0 + 1, a, a + 256)
                e_list.append(e_t)
            return o_ps, e_list

        def emit_attn_qb0_av(h, o_ps, e_list):
            po = (h % 2) * 64
            ch = h // 2
            # st-major accumulation: a start=True matmul clears the whole
            # PSUM bank's has_written bits, so each region's group must be
            # contiguous (no other start=True to this bank in between).
            for st in range(OCH):
                for kc in range(st + 1):
                    nc.tensor.matmul(
                        o_ps[:, st, :],
                        e_list[kc // 2][:, kc % 2, st * 128:(st + 1) * 128],
                        vv16[:, kc, h, :],
                        start=(kc == 0), stop=(kc == st))
            emit_divide_transpose(0, h, po, ch, o_ps)

        def emit_attn(qb, h):
            po = (h % 2) * 64
            ch = h // 2
            o_ps = op_pool.tile([128, OCH, D + 1], F32, tag="o")
            npair = (qb + 1) * 2
            e_list = []
            for pair in range(npair):
                kc0 = 2 * pair
                diag = kc0 - 4 * qb  # >=0 for diagonal pairs (0 or 2)
                s_ps = sp_pool.tile([128, 2, TB], F32, tag="s")
                a = 128 * diag if diag > 0 else 0
                for i in range(2):
                    kc = kc0 + i
                    co = a  # write the full exp window (no stale PSUM)
                    lhsT = k8[po:po + D, ch, :, kc * 128:(kc + 1) * 128]
                    rhs = q8[po:po + D, ch,
                             qb * TB + co:(qb + 1) * TB]
                    rhs = rhs.unsqueeze(1).broadcast_to([D, 2, TB - co])
                    nc.tensor.matmul(s_ps[:, i, co:], lhsT, rhs,
                                     start=True, stop=True, perf_mode=DR)
                if qb == 1:
                    e_t = e16_pool.tile([128, 2, TB], F16, tag="e")
                else:
                    e_t = e8_pool.tile([128, 2, TB], FP8E, tag="e")
                nc.scalar.activation(e_t[:, :, a:], s_ps[:, :, a:],
                                     AF.Exp, scale=0.125, bias=nbias[:])
                if diag >= 0:
                    affine_mask(e_t, 0, diag, a, a + 128)
                    affine_mask(e_t, 1, diag + 1, a, a + 256)
                e_list.append(e_t)
            # st-major accumulation (see emit_attn_qb0 comment)
            if qb == 1:
                # fp16 AV per chunk (8 key-chunks live in vv16)
                for st in range(OCH):
                    kcs = [kc for kc in range(4 * qb + st + 1)
                           if kc < 4 * qb or (kc - 4 * qb) <= st]
                    for n_, kc in enumerate(kcs):
                        nc.tensor.matmul(
                            o_ps[:, st, :],
                            e_list[kc // 2][:, kc % 2,
                                            st * 128:(st + 1) * 128],
                            vv16[:, kc, h, :],
                            start=(n_ == 0), stop=(n_ == len(kcs) - 1))
                emit_divide_transpose(qb, h, po, ch, o_ps)
                return
            for st in range(OCH):
                last = min(npair - 1, 2 * qb + st // 2)
                for pair in range(last + 1):
                    diag = 2 * pair - 4 * qb
                    if diag > 0 and st < diag:
                        continue
                    nc.tensor.matmul(
                        o_ps[:, st, :],
                        e_list[pair][:, :, st * 128:(st + 1) * 128],
                        vv8[:, 2 * pair:2 * pair + 2, h, :],
                        start=(pair == 0), stop=(pair == last),
                        perf_mode=DR)
            emit_divide_transpose(qb, h, po, ch, o_ps)

        def emit_divide_transpose(qb, h, po, ch, o_ps):
            r = rsb_pool.tile([128, OCH], F32, tag="r")
            nc.vector.reciprocal(r[:], o_ps[:, :, D])
            if po == 0:
                o2 = osb_pool.tile([128, OCH, 128], F16, tag="o2")
                _o2_live[ch] = o2
            else:
                o2 = _o2_live[ch]
            rb = r[:].unsqueeze(2).broadcast_to([128, OCH, D])
            nc.vector.tensor_mul(o2[:, :, po:po + D], o_ps[:, :, 0:D], rb)
            if po == 64:
                for st in range(OCH):
                    nc.sync.dma_start_transpose(
                        out=oT[:, ch,
                               qb * TB + st * 128:qb * TB + (st + 1) * 128],
                        in_=o2[:, st, :])

        def emit_yproj(qb, st, eh, tail=False):
            tt = qb * OCH + st
            tsl = slice(tt * 128, (tt + 1) * 128)
            y_ps = sc_pool.tile([128, 512], F32, tag="pp")
            for c in range(OCH):
                nc.tensor.matmul(
                    y_ps[:], oT[:, c, tsl],
                    wo_sb[:, c, eh * 512:(eh + 1) * 512],
                    start=(c == 0), stop=(c == OCH - 1))
            y_sb = ysb_pool.tile([128, 512], F16, tag="ysb")
            if tail:
                nc.scalar.copy(y_sb[:], y_ps[:])
            else:
                nc.vector.tensor_copy(y_sb[:], y_ps[:])
            nc.sync.dma_start(
                out=y[tsl, eh * 512:(eh + 1) * 512], in_=y_sb[:])

        # --------- schedule ---------
        _o2_live = {}
        # Block 0: project (k,q) per chunk and immediately run the two
        # heads' QK+exp so ACT starts early; v tiles follow ch0; AV waits
        # on v naturally via deps.
        for ch in range(OCH):
            emit_qk_proj(0, "k", ch)
            emit_qk_proj(0, "q", ch)
            emit_vproj(ch)  # v key-chunks 0..3 (t-block 0)
        for h in range(HPC):
            emit_attn_qb0_av(h, *emit_attn_qb0_qkexp(h))

        # pending work queues for interleaving
        pending_proj = []
        pending_y = []

        def drain_one_proj():
            if pending_proj:
                kind, arg = pending_proj.pop(0)
                if kind == "qk":
                    emit_qk_proj(*arg)
                else:
                    emit_vproj(arg)

        pending_proj = (
            [("v", 4 + i) for i in range(4)]
            + [("qk", (1, w, ch)) for ch in range(OCH) for w in ("k", "q")]
        )
        while pending_proj:
            drain_one_proj()
        pending_y = [(0, st, eh) for st in range(OCH) for eh in range(2)]

        for qb in range(1, NTB):
            nb = qb + 1  # next block to project during this attention
            if nb < NTB:
                pending_proj = (
                    [("qk", (nb, "k", 0)), ("qk", (nb, "q", 0))]
                    + [("v", nb * 4 + i) for i in range(4)]
                    + [("qk", (nb, w, ch)) for ch in range(1, OCH)
                       for w in ("k", "q")]
                )
            for h in range(HPC):
                emit_attn(qb, h)
                # interleave next-block projections + prev-qb out-proj;
                # front-load so next block's rope lands early
                for _ in range(4 if h < 3 else 1):
                    drain_one_proj()
                if pending_y:
                    emit_yproj(*pending_y.pop(0))
            while pending_proj:
                drain_one_proj()
            while pending_y:
                emit_yproj(*pending_y.pop(0))
            pending_y = [(qb, st, eh) for st in range(OCH)
                         for eh in range(2)]
        for args in pending_y:
            emit_yproj(*args, tail=True)

        if DEBUG_DUMP:
            dumps = [("d_q16", q16, F16), ("d_k16", k16, F16),
                     ("d_q8", q8, FP8), ("d_k8", k8, FP8),
                     ("d_vv8", vv8, FP8), ("d_vv16", vv16, F16),
                     ("d_oT", oT, F16)]
            for nm, t_, dt_ in dumps:
                shp = list(t_.shape)
                dd = nc.dram_tensor(nm, shp, dt_, kind="ExternalOutput").ap()
                nc.sync.dma_start(out=dd, in_=t_[:])

    nc.compile()
    return nc


def _host_inputs(x, Wq, Wk, Wv, Wo):
    inv_freq = 1.0 / (ROPE_BASE ** (np.arange(0, D, 2, dtype=np.float64) / D))
    freqs = np.outer(np.arange(T, dtype=np.float64), inv_freq)  # [T, 32]
    emb = np.concatenate([freqs, freqs], axis=-1)               # [T, 64]
    cos1, sin1 = np.cos(emb).T, np.sin(emb).T                   # [64, T]
    cosb = np.concatenate([cos1, cos1], 0).astype(np.float16)
    sinb = np.concatenate([sin1, sin1], 0).astype(np.float16)

    # rotate-half as lhsT: out = p2t.T @ q = R2 @ q
    R = np.zeros((64, 64), dtype=np.float32)
    for i in range(32):
        R[i, i + 32] = -1.0
        R[i + 32, i] = 1.0
    R2 = np.zeros((128, 128), dtype=np.float32)
    R2[0:64, 0:64] = R
    R2[64:128, 64:128] = R
    p2t = np.ascontiguousarray(R2.T).astype(np.float16)

    f8 = ml_dtypes.float8_e4m3
    xT = [np.ascontiguousarray(x[b].T) for b in range(B)]
    x8s = [t.astype(f8) for t in xT]
    x16s = [t[:, :TB].astype(np.float16) for t in xT]
    wmaps = []
    for hg in range(2):
        cols = slice(hg * EC, (hg + 1) * EC)
        wqT = np.ascontiguousarray(Wq[cols, :].T)
        wkT = np.ascontiguousarray(Wk[cols, :].T)
        wvT = np.ascontiguousarray(Wv[cols, :].T)
        woT = np.ascontiguousarray(Wo[:, cols].T)
        wmaps.append({
            "wq8": wqT.astype(f8), "wk8": wkT.astype(f8),
            "wv8": wvT.astype(f8),
            "wq16": wqT.astype(np.float16), "wk16": wkT.astype(np.float16),
            "wv16": wvT.astype(np.float16),
            "wo16": woT.astype(np.float16),
        })
    in_maps = []
    for c in range(N_CORES):
        b, hg = c // 2, c % 2
        in_maps.append({
            "x8": x8s[b], "x16": x16s[b], "p2t": p2t,
            "cosb": cosb, "sinb": sinb,
            **wmaps[hg],
        })
    return in_maps


def kernel(x, causal_mask, Wq, Wk, Wv, Wo):
    global _NC
    x = np.asarray(x, dtype=np.float32)
    Wq = np.asarray(Wq, dtype=np.float32)
    Wk = np.asarray(Wk, dtype=np.float32)
    Wv = np.asarray(Wv, dtype=np.float32)
    Wo = np.asarray(Wo, dtype=np.float32)
    if _NC is None:
        _NC = _build()
    in_maps = _host_inputs(x, Wq, Wk, Wv, Wo)
    try:
        res = run_bass_kernel_spmd(_NC, in_maps, list(range(N_CORES)))
    except Exception:
        import time
        time.sleep(2)
        res = run_bass_kernel_spmd(_NC, in_maps, list(range(N_CORES)))
    out = np.empty((B, T, E), dtype=np.float32)
    for b in range(B):
        out[b] = (res.results[2 * b]["y"].astype(np.float32)
                  + res.results[2 * b + 1]["y"].astype(np.float32))
    return out
